# revision 21
# baseline (speedup 1.0000x reference)
"""Distributed Trainium2 kernel for nn_Attention_60584808677611.

Head-sharded tensor parallelism over 8 NeuronCores: 2 heads per core.
v2: fused rstd+projection pass (single x stream keeps PE warm), AV
matmuls carry a ones-column (M=65) so softmax denominators ride free in
PSUM row 64, double-buffered S^T score tiles so exp overlaps the next
scores matmul, per-batch AllToAll overlapped with the other batch's
attention, batched output DMAs.
"""
import os
import numpy as np
import ml_dtypes

import concourse.bacc as bacc
import concourse.tile as tile
from concourse import mybir
from concourse.bass_utils import run_bass_kernel_spmd

F32 = mybir.dt.float32
BF16 = mybir.dt.bfloat16
F16 = mybir.dt.float16

B, N, DIM, H, DH, M = 2, 2048, 1024, 16, 64, 2048
NC = 8          # cores
HL = 2          # heads per core
T = B * N       # 4096 global tokens
KT = DIM // 128  # 8 contraction tiles
NCH = T // 512   # 8 token chunks of 512
MT = 33          # m tiles: 16 xl + 16 cur + 1 null
EPS = 1e-6
ROT = ((0, 32), (32, 0), (64, 96), (96, 64))  # rotate_half partition swaps

_CACHE = {}


def _build():
    nc = bacc.Bacc(None, num_devices=NC)
    dp = nc.declare_dram_parameter
    dbg = os.environ.get("KDBG") == "1"

    X_T = dp("x_t", [DIM, T], BF16, isOutput=False)
    WQ = dp("wq", [KT, 128, 128], BF16, isOutput=False)
    WK = dp("wk", [KT, 128, 128], BF16, isOutput=False)
    WVM = dp("wvm", [KT, 128, 130], BF16, isOutput=False)
    WOUT = dp("wout", [KT, 128, DIM], BF16, isOutput=False)
    COS_Q = dp("cos_q", [128, N], F16, isOutput=False)
    SIN_Q = dp("sin_q", [128, N], F16, isOutput=False)
    COS_K = dp("cos_k", [128, 2 * N], F16, isOutput=False)
    SIN_K = dp("sin_k", [128, 2 * N], F16, isOutput=False)
    XLK_T = dp("xlk_t", [B, 128, M], BF16, isOutput=False)
    XLVA = dp("xlva", [B, HL, M, 65], BF16, isOutput=False)
    NULLK_T = dp("nullk_t", [128, 1], BF16, isOutput=False)
    NULLVA = dp("nullva", [1, 130], BF16, isOutput=False)
    VR = dp("vr", [B, N, 128], F32, isOutput=False)
    ID_BF = dp("ident_bf", [128, 128], BF16, isOutput=False)
    ID_F32 = dp("ident_f32", [128, 128], F32, isOutput=False)

    # packed outputs: last dim = [h0 d | h1 d]
    K_PRE = dp("k_pre", [B, N, 128], F32, isOutput=True)
    V_MIX = dp("v_mix", [B, N, 128], F32, isOutput=True)
    V_ORIG = dp("v_orig", [B, N, 128], F32, isOutput=True)
    OUT_SH = dp("out_shard", [512, DIM], F32, isOutput=True)
    if dbg:
        DBG_ON = dp("dbg_onorm", [128, T], F32, isOutput=True)

    # per-batch a2a: core j owns tokens j*256.. within each batch
    a2a_in = [nc.dram_tensor(f"a2a_in{b}", [NC, 128, 256], BF16)
              for b in range(B)]
    a2a_out = [nc.dram_tensor(f"a2a_out{b}", [NC, 128, 256], BF16)
               for b in range(B)]

    x_t_3d = X_T.ap().rearrange("(kt p) t -> p kt t", p=128)

    with tile.TileContext(nc) as tc:
        with tc.tile_pool(name="persist", bufs=1) as pp:
            wq_sb = pp.tile([128, KT, 128], BF16)
            wk_sb = pp.tile([128, KT, 128], BF16)
            wvm_sb = pp.tile([128, KT, 130], BF16)
            wout_sb = pp.tile([128, KT, DIM], BF16)
            cosq_sb = pp.tile([128, N], F16)
            sinq_sb = pp.tile([128, N], F16)
            cosk_sb = pp.tile([128, 2 * N], F16)
            sink_sb = pp.tile([128, 2 * N], F16)
            nullk_sb = pp.tile([128, 1], BF16)
            nullva_sb = pp.tile([1, 130], BF16)
            id_bf_sb = pp.tile([128, 128], BF16)
            id_f32_sb = pp.tile([128, 128], F32)
            ones_bf = pp.tile([128, 1], BF16)
            ones_f32 = pp.tile([1, 128], F32)
            eps_col = pp.tile([128, 1], F32)
            q_T = pp.tile([128, T], BF16)
            kT_b = [pp.tile([128, 2 * N], BF16, name=f"kT{b}", tag=f"kT{b}")
                    for b in range(B)]
            xlv_sb = [[pp.tile([128, 16 * 65], BF16, name=f"xlv{b}{h}",
                               tag=f"xlv{b}{h}")
                       for h in range(HL)] for b in range(B)]
            vm_sb = [[pp.tile([128, 16 * 65], BF16, name=f"vm{b}{h}",
                              tag=f"vm{b}{h}")
                      for h in range(HL)] for b in range(B)]
            o_norm = pp.tile([128, T], BF16)
            oall_sb = [pp.tile([128, NC, 256], BF16, name=f"oall{b}",
                               tag=f"oall{b}") for b in range(B)]

            nc.sync.dma_start(wq_sb[:], WQ.ap().transpose([1, 0, 2]))
            nc.sync.dma_start(wk_sb[:], WK.ap().transpose([1, 0, 2]))
            nc.sync.dma_start(wvm_sb[:], WVM.ap().transpose([1, 0, 2]))
            nc.sync.dma_start(wout_sb[:], WOUT.ap().transpose([1, 0, 2]))
            nc.sync.dma_start(cosq_sb[:], COS_Q[:])
            nc.sync.dma_start(sinq_sb[:], SIN_Q[:])
            nc.sync.dma_start(cosk_sb[:], COS_K[:])
            nc.sync.dma_start(sink_sb[:], SIN_K[:])
            nc.sync.dma_start(nullk_sb[:], NULLK_T[:])
            nc.sync.dma_start(nullva_sb[:], NULLVA[:])
            nc.sync.dma_start(id_bf_sb[:], ID_BF[:])
            nc.sync.dma_start(id_f32_sb[:], ID_F32[:])
            for b in range(B):
                nc.sync.dma_start(kT_b[b][:, 0:M], XLK_T[b])
                for h in range(HL):
                    nc.sync.dma_start(
                        xlv_sb[b][h][:].rearrange("p (mt d) -> p mt d", d=65),
                        XLVA[b, h].rearrange("(mt p) d -> p mt d", p=128))
                    nc.vector.memset(vm_sb[b][h][:], 1.0)
            nc.vector.memset(ones_bf[:], 1.0)
            nc.vector.memset(ones_f32[:], 1.0)
            nc.vector.memset(eps_col[:], EPS)

            # ---------- Phase AB: fused rstd + projections ----------
            with tc.tile_pool(name="pb_sb", bufs=2) as pb, \
                 tc.tile_pool(name="pb_ps", bufs=2, space="PSUM") as pb_ps, \
                 tc.tile_pool(name="pb_ps1", bufs=1, space="PSUM") as pb_ps1:
                for ch in range(NCH):
                    b = ch // 4
                    nn = (ch % 4) * 512        # position within batch
                    c0 = ch * 512              # global token offset
                    xch = pb.tile([128, KT, 512], BF16, tag="xch", bufs=3)
                    nc.sync.dma_start(xch[:], x_t_3d[:, :, c0:c0 + 512])

                    # rstd for this chunk
                    ms_ps = pb_ps1.tile([1, 512], F32, tag="ms")
                    for kt in range(KT):
                        xsq = pb.tile([128, 512], BF16, tag="xsq")
                        nc.vector.tensor_mul(xsq[:], xch[:, kt], xch[:, kt])
                        nc.tensor.matmul(ms_ps[:], ones_bf[:], xsq[:],
                                         start=(kt == 0), stop=(kt == KT - 1))
                    ms_sb = pb.tile([1, 512], F32, tag="mssb")
                    nc.vector.tensor_copy(ms_sb[:], ms_ps[:])
                    msb_ps = pb_ps1.tile([128, 512], F32, tag="msb")
                    nc.tensor.matmul(msb_ps[:], ones_f32[:], ms_sb[:],
                                     start=True, stop=True)
                    sq_sb = pb.tile([128, 512], F32, tag="sqsb")
                    nc.scalar.activation(sq_sb[:], msb_ps[:],
                                         mybir.ActivationFunctionType.Sqrt,
                                         scale=1.0 / DIM, bias=eps_col[:])
                    rch = pb.tile([128, 512], F32, tag="rch")
                    nc.vector.reciprocal(rch[:], sq_sb[:])
                    rcol = pb.tile([128, 4], F32, tag="rcol")
                    for j in range(4):
                        rt_ps = pb_ps1.tile([128, 128], F32, tag="tps")
                        nc.tensor.transpose(
                            rt_ps[:].bitcast(F32),
                            rch[:, j * 128:(j + 1) * 128], id_f32_sb[:])
                        nc.vector.tensor_copy(rcol[:, j:j + 1], rt_ps[:, 0:1])

                    # --- q ---
                    q_ps = pb_ps.tile([128, 512], F32, tag="qps")
                    for kt in range(KT):
                        nc.tensor.matmul(q_ps[:], wq_sb[:, kt], xch[:, kt],
                                         start=(kt == 0), stop=(kt == KT - 1))
                    qn = pb.tile([128, 512], F32, tag="qn")
                    nc.vector.tensor_mul(qn[:], q_ps[:], rch[:])
                    qrot = pb.tile([128, 512], F32, tag="qrot")
                    for (d, s) in ROT:
                        nc.sync.dma_start(qrot[d:d + 32, :], qn[s:s + 32, :])
                    qa = pb.tile([128, 512], F32, tag="qa")
                    nc.vector.tensor_mul(qa[:], qn[:], cosq_sb[:, nn:nn + 512])
                    qb = pb.tile([128, 512], F32, tag="qb")
                    nc.vector.tensor_mul(qb[:], qrot[:], sinq_sb[:, nn:nn + 512])
                    nc.vector.tensor_add(q_T[:, c0:c0 + 512], qa[:], qb[:])

                    # --- k ---
                    k_ps = pb_ps.tile([128, 512], F32, tag="kps")
                    for kt in range(KT):
                        nc.tensor.matmul(k_ps[:], wk_sb[:, kt], xch[:, kt],
                                         start=(kt == 0), stop=(kt == KT - 1))
                    kn = pb.tile([128, 512], F32, tag="kn")
                    nc.vector.tensor_mul(kn[:], k_ps[:], rch[:])
                    kpre = pb.tile([128, 512], BF16, tag="kpre")
                    nc.vector.tensor_copy(kpre[:], kn[:])
                    kt_sb = pb.tile([128, 4, 128], BF16, tag="ktsb")
                    for j in range(4):
                        kt_ps = pb_ps1.tile([128, 128], BF16, tag="tps")
                        nc.tensor.transpose(kt_ps[:],
                                            kpre[:, j * 128:(j + 1) * 128],
                                            id_bf_sb[:])
                        nc.vector.tensor_copy(kt_sb[:, j], kt_ps[:])
                    nc.gpsimd.dma_start(
                        K_PRE[b, nn:nn + 512, :].rearrange(
                            "(j p) d -> p j d", p=128), kt_sb[:])
                    krot = pb.tile([128, 512], F32, tag="krot")
                    for (d, s) in ROT:
                        nc.sync.dma_start(krot[d:d + 32, :], kn[s:s + 32, :])
                    ka = pb.tile([128, 512], F32, tag="ka")
                    nc.vector.tensor_mul(ka[:], kn[:],
                                         cosk_sb[:, M + nn:M + nn + 512])
                    kb = pb.tile([128, 512], F32, tag="kb")
                    nc.vector.tensor_mul(kb[:], krot[:],
                                         sink_sb[:, M + nn:M + nn + 512])
                    nc.vector.tensor_add(kT_b[b][:, M + nn:M + nn + 512],
                                         ka[:], kb[:])

                    # --- v + mix (token-major) ---
                    for s4 in range(4):
                        nblk = nn // 128 + s4      # 0..15 within batch
                        v_ps = pb_ps.tile([128, 130], F32, tag="vps", bufs=1)
                        for kt in range(KT):
                            nc.tensor.matmul(
                                v_ps[:], xch[:, kt, s4 * 128:(s4 + 1) * 128],
                                wvm_sb[:, kt], start=(kt == 0),
                                stop=(kt == KT - 1))
                        vs = pb.tile([128, 130], F32, tag="vs")
                        nc.vector.tensor_scalar(vs[:], v_ps[:],
                                                rcol[:, s4:s4 + 1],
                                                None, mybir.AluOpType.mult)
                        nc.sync.dma_start(
                            V_ORIG[b, nblk * 128:(nblk + 1) * 128, :],
                            vs[:, 0:128])
                        th = pb.tile([128, 2], F32, tag="th")
                        nc.scalar.activation(th[:], vs[:, 128:130],
                                             mybir.ActivationFunctionType.Tanh,
                                             scale=0.5)
                        mix = pb.tile([128, 2], F32, tag="mix")
                        nc.vector.tensor_scalar(mix[:], th[:], 0.5, 0.5,
                                                mybir.AluOpType.mult,
                                                mybir.AluOpType.add)
                        vr_t = pb.tile([128, 128], F32, tag="vrt")
                        nc.sync.dma_start(
                            vr_t[:], VR[b, nblk * 128:(nblk + 1) * 128, :])
                        vm128 = pb.tile([128, 128], BF16, tag="vm128")
                        for h in range(HL):
                            d1 = pb.tile([128, DH], F32, tag="d1")
                            nc.vector.tensor_sub(d1[:],
                                                 vr_t[:, h * 64:(h + 1) * 64],
                                                 vs[:, h * 64:(h + 1) * 64])
                            d2 = pb.tile([128, DH], F32, tag="d2")
                            nc.vector.tensor_scalar(d2[:], d1[:],
                                                    mix[:, h:h + 1], None,
                                                    mybir.AluOpType.mult)
                            vmt = vm_sb[b][h][:, nblk * 65:nblk * 65 + 64]
                            nc.vector.tensor_add(vmt,
                                                 vs[:, h * 64:(h + 1) * 64],
                                                 d2[:])
                            nc.vector.tensor_copy(vm128[:, h * 64:(h + 1) * 64],
                                                  vmt)
                        nc.gpsimd.dma_start(
                            V_MIX[b, nblk * 128:(nblk + 1) * 128, :], vm128[:])

                # --- xl k rope (in place on kT_b[:, 0:M]) ---
                for b in range(B):
                    for ch4 in range(4):
                        cc = ch4 * 512
                        xrot = pb.tile([128, 512], BF16, tag="xrot")
                        for (d, s) in ROT:
                            nc.sync.dma_start(xrot[d:d + 32, :],
                                              kT_b[b][s:s + 32, cc:cc + 512])
                        xa = pb.tile([128, 512], F32, tag="xa")
                        nc.vector.tensor_mul(xa[:], kT_b[b][:, cc:cc + 512],
                                             cosk_sb[:, cc:cc + 512])
                        xb = pb.tile([128, 512], F32, tag="xb")
                        nc.vector.tensor_mul(xb[:], xrot[:],
                                             sink_sb[:, cc:cc + 512])
                        nc.vector.tensor_add(kT_b[b][:, cc:cc + 512],
                                             xa[:], xb[:])

            # ---------- Phase C: attention + per-batch a2a/outproj ----------
            with tc.tile_pool(name="pc_sb", bufs=2) as pc, \
                 tc.tile_pool(name="pc_ps", bufs=1, space="PSUM") as pcp:
                for b in range(B):
                    for nt in range(4):
                        n0 = b * N + nt * 512   # global token offset
                        o_ps = [pcp.tile([65, 512], F32, name=f"oh{h}",
                                         tag=f"oh{h}") for h in range(HL)]
                        for m in range(MT):
                            mlen = 128 if m < 32 else 1
                            s_ps = pcp.tile([128, 1024], F32, tag="s", bufs=2)
                            for h in range(HL):
                                if m < 32:
                                    lk = kT_b[b][h * 64:(h + 1) * 64,
                                                 m * 128:m * 128 + 128]
                                else:
                                    lk = nullk_sb[h * 64:(h + 1) * 64, 0:1]
                                rq = q_T[h * 64:(h + 1) * 64, n0:n0 + 512]
                                nc.tensor.matmul(
                                    s_ps[0:mlen, h * 512:(h + 1) * 512],
                                    lk, rq, start=True, stop=True,
                                    tile_position=(h * 64, 0))
                            e = pc.tile([128, 1024], BF16, tag="exps", bufs=4)
                            nc.scalar.activation(
                                e[0:mlen, :], s_ps[0:mlen, :],
                                mybir.ActivationFunctionType.Exp)
                            for h in range(HL):
                                if m < 16:
                                    lv = xlv_sb[b][h][0:mlen,
                                                      m * 65:(m + 1) * 65]
                                elif m < 32:
                                    lv = vm_sb[b][h][0:mlen,
                                                     (m - 16) * 65:
                                                     (m - 15) * 65]
                                else:
                                    lv = nullva_sb[0:1, h * 65:(h + 1) * 65]
                                nc.tensor.matmul(
                                    o_ps[h][:],
                                    lv, e[0:mlen, h * 512:(h + 1) * 512],
                                    start=(m == 0), stop=(m == MT - 1),
                                    skip_group_check=True)
                        for h in range(HL):
                            rec = pc.tile([1, 512], F32, tag="rec")
                            nc.vector.reciprocal(rec[:], o_ps[h][64:65, :])
                            bc_ps = pcp.tile([64, 512], F32, tag="bc")
                            nc.tensor.matmul(bc_ps[:], ones_f32[0:1, 0:64],
                                             rec[:], start=True, stop=True)
                            bc_sb = pc.tile([64, 512], F32, tag="bcsb")
                            nc.vector.tensor_copy(bc_sb[:], bc_ps[:])
                            nc.vector.tensor_mul(
                                o_norm[h * 64:(h + 1) * 64, n0:n0 + 512],
                                o_ps[h][0:64, :], bc_sb[:])
                    # a2a for this batch fires immediately (comm overlaps
                    # the next batch's attention); out-projection deferred
                    for j in range(NC):
                        nc.gpsimd.dma_start(
                            a2a_in[b][j],
                            o_norm[:, b * N + j * 256:b * N + (j + 1) * 256])
                    nc.gpsimd.collective_compute(
                        "AllToAll", mybir.AluOpType.bypass,
                        replica_groups=[list(range(NC))],
                        ins=[a2a_in[b].ap().opt()],
                        outs=[a2a_out[b].ap().opt()])
                    nc.gpsimd.dma_start(oall_sb[b][:],
                                        a2a_out[b].ap().transpose([1, 0, 2]))
                    if dbg:
                        nc.gpsimd.dma_start(DBG_ON[:, b * N:(b + 1) * N],
                                            o_norm[:, b * N:(b + 1) * N])
                # out projections after all attention: b0's data has long
                # arrived; only b1's collective can expose latency
                for b in range(B):
                    for tch in range(2):
                        for dch in range(2):
                            op = pcp.tile([128, 512], F32, tag="op")
                            for r in range(NC):
                                nc.tensor.matmul(
                                    op[:],
                                    oall_sb[b][:, r, tch * 128:(tch + 1) * 128],
                                    wout_sb[:, r, dch * 512:(dch + 1) * 512],
                                    start=(r == 0), stop=(r == NC - 1))
                            os_sb = pc.tile([128, 512], F32, tag="os")
                            nc.vector.tensor_copy(os_sb[:], op[:])
                            nc.sync.dma_start(
                                OUT_SH[b * 256 + tch * 128:
                                       b * 256 + (tch + 1) * 128,
                                       dch * 512:(dch + 1) * 512], os_sb[:])
    nc.finalize()
    return nc


def _rope_np(pos, t):
    t1, t2 = t[..., :32], t[..., 32:]
    rot = np.concatenate((-t2, t1), axis=-1)
    return t * np.cos(pos) + rot * np.sin(pos)


def _prep(inputs):
    bf = ml_dtypes.bfloat16
    x = np.asarray(inputs["x"], np.float32)
    vr = np.asarray(inputs["value_residual"], np.float32)
    xlm = np.asarray(inputs["xl_memories"], np.float32)
    rq = np.asarray(inputs["rotary_q"], np.float32)
    rk = np.asarray(inputs["rotary_k"], np.float32)
    nw = np.asarray(inputs["norm_w"], np.float32)
    Wq = np.asarray(inputs["Wq"], np.float32)
    Wkv = np.asarray(inputs["Wkv"], np.float32)
    Wout = np.asarray(inputs["Wout"], np.float32)
    nkv = np.asarray(inputs["null_kv"], np.float32)
    Wmix = np.asarray(inputs["Wmix"], np.float32)

    x_t = np.ascontiguousarray(x.reshape(T, DIM).T).astype(bf)

    scale = DH ** -0.5
    Wq_n = nw[:, None] * Wq * scale
    Wk_n = nw[:, None] * Wkv[:, :H * DH]
    Wv_n = nw[:, None] * Wkv[:, H * DH:]
    Wmix_n = nw[:, None] * Wmix

    cq = np.cos(rq).T
    sq = np.sin(rq).T.copy()
    sq[:32] *= -1.0
    cq = np.tile(cq, (2, 1)).astype(np.float16)
    sq = np.tile(sq, (2, 1)).astype(np.float16)

    # kv order on device: [xl (rk[0:M]) | cur (rk[1+M:])]; null uses rk[M]
    pos_k = np.concatenate([rk[0:M], rk[1 + M:1 + M + N]], axis=0)
    ck = np.cos(pos_k).T
    sk = np.sin(pos_k).T.copy()
    sk[:32] *= -1.0
    ck = np.tile(ck, (2, 1)).astype(np.float16)
    sk = np.tile(sk, (2, 1)).astype(np.float16)

    nk_roped = _rope_np(rk[M], nkv[0])     # [16, 64]
    ident = np.eye(128, dtype=np.float32)
    wout_t = np.ascontiguousarray(Wout.reshape(KT, 128, DIM)).astype(bf)

    ones_m = np.ones((B, HL, M, 1), np.float32)
    in_maps = []
    for c in range(NC):
        cols = slice(c * 128, (c + 1) * 128)
        hc = slice(c * HL, c * HL + HL)
        wq_c = np.ascontiguousarray(
            Wq_n[:, cols].reshape(KT, 128, 128)).astype(bf)
        wk_c = np.ascontiguousarray(
            Wk_n[:, cols].reshape(KT, 128, 128)).astype(bf)
        wvm_c = np.ascontiguousarray(
            np.concatenate([Wv_n[:, cols], Wmix_n[:, hc]], axis=1)
            .reshape(KT, 128, 130)).astype(bf)
        xlk_c = np.ascontiguousarray(
            xlm[0][:, hc].transpose(0, 1, 3, 2).reshape(B, 128, M)).astype(bf)
        xlva_c = np.ascontiguousarray(
            np.concatenate([xlm[1][:, hc], ones_m], axis=3)).astype(bf)
        vr_c = np.ascontiguousarray(
            vr[:, hc].transpose(0, 2, 1, 3).reshape(B, N, 128)
        ).astype(np.float32)
        nullk_c = np.ascontiguousarray(nk_roped[hc].reshape(128, 1)).astype(bf)
        nullva_c = np.concatenate(
            [nkv[1][c * HL], [1.0], nkv[1][c * HL + 1], [1.0]]
        ).reshape(1, 130).astype(bf)
        in_maps.append(dict(
            x_t=x_t, wq=wq_c, wk=wk_c, wvm=wvm_c, wout=wout_t,
            cos_q=cq, sin_q=sq, cos_k=ck, sin_k=sk,
            xlk_t=xlk_c, xlva=xlva_c, nullk_t=nullk_c, nullva=nullva_c,
            vr=vr_c, ident_bf=ident.astype(bf), ident_f32=ident))
    return in_maps


def kernel(**inputs):
    if "nc" not in _CACHE:
        _CACHE["nc"] = _build()
    nc = _CACHE["nc"]
    in_maps = _prep(inputs)
    res = run_bass_kernel_spmd(nc, in_maps, core_ids=list(range(NC)))
    rs = res.results

    # out: per batch, core c owns tokens c*256..(c+1)*256
    out = np.empty((B, N, DIM), np.float32)
    for c in range(NC):
        sh = rs[c]["out_shard"]
        for b in range(B):
            out[b, c * 256:(c + 1) * 256] = sh[b * 256:(b + 1) * 256]

    def unpack(name):
        # [B, N, 128] per core -> [B, HL, N, DH] -> concat heads
        parts = [rs[c][name].reshape(B, N, HL, DH).transpose(0, 2, 1, 3)
                 for c in range(NC)]
        return np.concatenate(parts, axis=1)

    k_pre = unpack("k_pre")
    v_mix = unpack("v_mix")
    v_orig = unpack("v_orig")

    nkv = np.asarray(inputs["null_kv"], np.float32)
    nk = np.broadcast_to(nkv[0][None, :, None, :], (B, H, 1, DH))
    nv = np.broadcast_to(nkv[1][None, :, None, :], (B, H, 1, DH))
    next_k = np.concatenate([nk, k_pre], axis=2)
    next_v = np.concatenate([nv, v_mix], axis=2)
    next_xl = np.stack([next_k, next_v]).astype(np.float32)
    return out, next_xl, v_orig.astype(np.float32)


# revision 23
# speedup vs baseline: 1.0875x; 1.0875x over previous
"""Distributed Trainium2 kernel for nn_Attention_60584808677611.

Head-sharded tensor parallelism over 8 NeuronCores: 2 heads per core.
v2: fused rstd+projection pass (single x stream keeps PE warm), AV
matmuls carry a ones-column (M=65) so softmax denominators ride free in
PSUM row 64, double-buffered S^T score tiles so exp overlaps the next
scores matmul, per-batch AllToAll overlapped with the other batch's
attention, batched output DMAs.
"""
import os
import numpy as np
import ml_dtypes

import concourse.bacc as bacc
import concourse.tile as tile
from concourse import mybir
from concourse.bass_utils import run_bass_kernel_spmd

F32 = mybir.dt.float32
BF16 = mybir.dt.bfloat16
F16 = mybir.dt.float16

B, N, DIM, H, DH, M = 2, 2048, 1024, 16, 64, 2048
NC = 8          # cores
HL = 2          # heads per core
T = B * N       # 4096 global tokens
KT = DIM // 128  # 8 contraction tiles
NCH = T // 512   # 8 token chunks of 512
MT = 33          # m tiles: 16 xl + 16 cur + 1 null
EPS = 1e-6
ROT = ((0, 32), (32, 0), (64, 96), (96, 64))  # rotate_half partition swaps

_CACHE = {}


def _build():
    nc = bacc.Bacc(None, num_devices=NC)
    dp = nc.declare_dram_parameter
    dbg = os.environ.get("KDBG") == "1"

    X_T = dp("x_t", [DIM, T], BF16, isOutput=False)
    WQ = dp("wq", [KT, 128, 128], BF16, isOutput=False)
    WK = dp("wk", [KT, 128, 128], BF16, isOutput=False)
    WVM = dp("wvm", [KT, 128, 130], BF16, isOutput=False)
    WOUT = dp("wout", [KT, 128, DIM], BF16, isOutput=False)
    COS_Q = dp("cos_q", [128, N], F16, isOutput=False)
    SIN_Q = dp("sin_q", [128, N], F16, isOutput=False)
    COS_K = dp("cos_k", [128, 2 * N], F16, isOutput=False)
    SIN_K = dp("sin_k", [128, 2 * N], F16, isOutput=False)
    XLK_T = dp("xlk_t", [B, 128, M], BF16, isOutput=False)
    XLVA = dp("xlva", [B, HL, M, 65], BF16, isOutput=False)
    NULLK_T = dp("nullk_t", [128, 1], BF16, isOutput=False)
    NULLVA = dp("nullva", [1, 130], BF16, isOutput=False)
    VR = dp("vr", [B, N, 128], F32, isOutput=False)
    ID_BF = dp("ident_bf", [128, 128], BF16, isOutput=False)
    ID_F32 = dp("ident_f32", [128, 128], F32, isOutput=False)

    # packed outputs: last dim = [h0 d | h1 d]
    K_PRE = dp("k_pre", [B, N, 128], F32, isOutput=True)
    V_MIX = dp("v_mix", [B, N, 128], F32, isOutput=True)
    V_ORIG = dp("v_orig", [B, N, 128], F32, isOutput=True)
    OUT_SH = dp("out_shard", [512, DIM], F32, isOutput=True)
    if dbg:
        DBG_ON = dp("dbg_onorm", [128, T], F32, isOutput=True)

    # per-batch a2a: core j owns tokens j*256.. within each batch
    a2a_in = [nc.dram_tensor(f"a2a_in{b}", [NC, 128, 256], BF16)
              for b in range(B)]
    a2a_out = [nc.dram_tensor(f"a2a_out{b}", [NC, 128, 256], BF16)
               for b in range(B)]

    x_t_3d = X_T.ap().rearrange("(kt p) t -> p kt t", p=128)

    with tile.TileContext(nc) as tc:
        with tc.tile_pool(name="persist", bufs=1) as pp:
            wq_sb = pp.tile([128, KT, 128], BF16)
            wk_sb = pp.tile([128, KT, 128], BF16)
            wvm_sb = pp.tile([128, KT, 130], BF16)
            wout_sb = pp.tile([128, KT, DIM], BF16)
            cosq_sb = pp.tile([128, N], F16)
            sinq_sb = pp.tile([128, N], F16)
            cosk_sb = pp.tile([128, 2 * N], F16)
            sink_sb = pp.tile([128, 2 * N], F16)
            nullk_sb = pp.tile([128, 1], BF16)
            nullva_sb = pp.tile([1, 130], BF16)
            id_bf_sb = pp.tile([128, 128], BF16)
            id_f32_sb = pp.tile([128, 128], F32)
            ones_bf = pp.tile([128, 1], BF16)
            ones_f32 = pp.tile([1, 128], F32)
            eps_col = pp.tile([128, 1], F32)
            q_T = pp.tile([128, T], BF16)
            kT_b = [pp.tile([128, 2 * N], BF16, name=f"kT{b}", tag=f"kT{b}")
                    for b in range(B)]
            xlv_sb = [[pp.tile([128, 16 * 65], BF16, name=f"xlv{b}{h}",
                               tag=f"xlv{b}{h}")
                       for h in range(HL)] for b in range(B)]
            vm_sb = [[pp.tile([128, 16 * 65], BF16, name=f"vm{b}{h}",
                              tag=f"vm{b}{h}")
                      for h in range(HL)] for b in range(B)]
            o_norm = pp.tile([128, T], BF16)
            oall_sb = [pp.tile([128, NC, 256], BF16, name=f"oall{b}",
                               tag=f"oall{b}") for b in range(B)]

            nc.sync.dma_start(wq_sb[:], WQ.ap().transpose([1, 0, 2]))
            nc.sync.dma_start(wk_sb[:], WK.ap().transpose([1, 0, 2]))
            nc.sync.dma_start(wvm_sb[:], WVM.ap().transpose([1, 0, 2]))
            nc.sync.dma_start(wout_sb[:], WOUT.ap().transpose([1, 0, 2]))
            nc.sync.dma_start(cosq_sb[:], COS_Q[:])
            nc.sync.dma_start(sinq_sb[:], SIN_Q[:])
            nc.sync.dma_start(cosk_sb[:], COS_K[:])
            nc.sync.dma_start(sink_sb[:], SIN_K[:])
            nc.sync.dma_start(nullk_sb[:], NULLK_T[:])
            nc.sync.dma_start(nullva_sb[:], NULLVA[:])
            nc.sync.dma_start(id_bf_sb[:], ID_BF[:])
            nc.sync.dma_start(id_f32_sb[:], ID_F32[:])
            for b in range(B):
                nc.sync.dma_start(kT_b[b][:, 0:M], XLK_T[b])
                for h in range(HL):
                    nc.sync.dma_start(
                        xlv_sb[b][h][:].rearrange("p (mt d) -> p mt d", d=65),
                        XLVA[b, h].rearrange("(mt p) d -> p mt d", p=128))
                    nc.vector.memset(vm_sb[b][h][:], 1.0)
            nc.vector.memset(ones_bf[:], 1.0)
            nc.vector.memset(ones_f32[:], 1.0)
            nc.vector.memset(eps_col[:], EPS)

            # ---------- fused rstd+projection chunks (psum source varies) ----
            with tc.tile_pool(name="pb_sb", bufs=2) as pb, \
                 tc.tile_pool(name="pc_sb", bufs=2) as pc:

                def emit_chunk(ch, psget):
                    b = ch // 4
                    nn = (ch % 4) * 512        # position within batch
                    c0 = ch * 512              # global token offset
                    xch = pb.tile([128, KT, 512], BF16, tag="xch", bufs=2,
                                  name=f"xch{ch}")
                    nc.sync.dma_start(xch[:], x_t_3d[:, :, c0:c0 + 512])

                    ms_ps = psget([1, 512], F32, "ms")
                    for kt in range(KT):
                        xsq = pb.tile([128, 512], BF16, tag="xsq",
                                      name=f"xsq{ch}_{kt}")
                        nc.vector.tensor_mul(xsq[:], xch[:, kt], xch[:, kt])
                        nc.tensor.matmul(ms_ps[:], ones_bf[:], xsq[:],
                                         start=(kt == 0), stop=(kt == KT - 1))
                    ms_sb = pb.tile([1, 512], F32, tag="mssb", name=f"ms{ch}")
                    nc.vector.tensor_copy(ms_sb[:], ms_ps[:])
                    msb_ps = psget([128, 512], F32, "msb")
                    nc.tensor.matmul(msb_ps[:], ones_f32[:], ms_sb[:],
                                     start=True, stop=True)
                    sq_sb = pb.tile([128, 512], F32, tag="sqsb",
                                    name=f"sq{ch}")
                    nc.scalar.activation(sq_sb[:], msb_ps[:],
                                         mybir.ActivationFunctionType.Sqrt,
                                         scale=1.0 / DIM, bias=eps_col[:])
                    rch = pb.tile([128, 512], F32, tag="rch", name=f"rch{ch}")
                    nc.vector.reciprocal(rch[:], sq_sb[:])
                    rcol = pb.tile([128, 4], F32, tag="rcol", name=f"rc{ch}")
                    for j in range(4):
                        rt_ps = psget([128, 128], F32, "tps")
                        nc.tensor.transpose(
                            rt_ps[:].bitcast(F32),
                            rch[:, j * 128:(j + 1) * 128], id_f32_sb[:])
                        nc.vector.tensor_copy(rcol[:, j:j + 1], rt_ps[:, 0:1])

                    # --- q ---
                    q_ps = psget([128, 512], F32, "qps")
                    for kt in range(KT):
                        nc.tensor.matmul(q_ps[:], wq_sb[:, kt], xch[:, kt],
                                         start=(kt == 0), stop=(kt == KT - 1))
                    qn = pb.tile([128, 512], F32, tag="qn", name=f"qn{ch}")
                    nc.vector.tensor_mul(qn[:], q_ps[:], rch[:])
                    qrot = pb.tile([128, 512], F32, tag="qrot", name=f"qr{ch}")
                    for (d, sp) in ROT:
                        nc.sync.dma_start(qrot[d:d + 32, :], qn[sp:sp + 32, :])
                    qa = pb.tile([128, 512], F32, tag="qa", name=f"qa{ch}")
                    nc.vector.tensor_mul(qa[:], qn[:], cosq_sb[:, nn:nn + 512])
                    qb = pb.tile([128, 512], F32, tag="qb", name=f"qb{ch}")
                    nc.vector.tensor_mul(qb[:], qrot[:],
                                         sinq_sb[:, nn:nn + 512])
                    nc.vector.tensor_add(q_T[:, c0:c0 + 512], qa[:], qb[:])

                    # --- k ---
                    k_ps = psget([128, 512], F32, "kps")
                    for kt in range(KT):
                        nc.tensor.matmul(k_ps[:], wk_sb[:, kt], xch[:, kt],
                                         start=(kt == 0), stop=(kt == KT - 1))
                    kn = pb.tile([128, 512], F32, tag="kn", name=f"kn{ch}")
                    nc.vector.tensor_mul(kn[:], k_ps[:], rch[:])
                    kpre = pb.tile([128, 512], BF16, tag="kpre",
                                   name=f"kp{ch}")
                    nc.vector.tensor_copy(kpre[:], kn[:])
                    kt_sb = pb.tile([128, 4, 128], BF16, tag="ktsb",
                                    name=f"kt{ch}")
                    for j in range(4):
                        kt_ps = psget([128, 128], BF16, "tps")
                        nc.tensor.transpose(kt_ps[:],
                                            kpre[:, j * 128:(j + 1) * 128],
                                            id_bf_sb[:])
                        nc.vector.tensor_copy(kt_sb[:, j], kt_ps[:])
                    nc.gpsimd.dma_start(
                        K_PRE[b, nn:nn + 512, :].rearrange(
                            "(j p) d -> p j d", p=128), kt_sb[:])
                    krot = pb.tile([128, 512], F32, tag="krot", name=f"kr{ch}")
                    for (d, sp) in ROT:
                        nc.sync.dma_start(krot[d:d + 32, :], kn[sp:sp + 32, :])
                    ka = pb.tile([128, 512], F32, tag="ka", name=f"ka{ch}")
                    nc.vector.tensor_mul(ka[:], kn[:],
                                         cosk_sb[:, M + nn:M + nn + 512])
                    kb = pb.tile([128, 512], F32, tag="kb", name=f"kb{ch}")
                    nc.vector.tensor_mul(kb[:], krot[:],
                                         sink_sb[:, M + nn:M + nn + 512])
                    nc.vector.tensor_add(kT_b[b][:, M + nn:M + nn + 512],
                                         ka[:], kb[:])

                    # --- v + mix (token-major) ---
                    for s4 in range(4):
                        nblk = nn // 128 + s4
                        v_ps = psget([128, 130], F32, "vps")
                        for kt in range(KT):
                            nc.tensor.matmul(
                                v_ps[:], xch[:, kt, s4 * 128:(s4 + 1) * 128],
                                wvm_sb[:, kt], start=(kt == 0),
                                stop=(kt == KT - 1))
                        vs = pb.tile([128, 130], F32, tag="vs",
                                     name=f"vs{ch}_{s4}")
                        nc.vector.tensor_scalar(vs[:], v_ps[:],
                                                rcol[:, s4:s4 + 1],
                                                None, mybir.AluOpType.mult)
                        nc.sync.dma_start(
                            V_ORIG[b, nblk * 128:(nblk + 1) * 128, :],
                            vs[:, 0:128])
                        th = pb.tile([128, 2], F32, tag="th",
                                     name=f"th{ch}_{s4}")
                        nc.scalar.activation(th[:], vs[:, 128:130],
                                             mybir.ActivationFunctionType.Tanh,
                                             scale=0.5)
                        mix = pb.tile([128, 2], F32, tag="mix",
                                      name=f"mx{ch}_{s4}")
                        nc.vector.tensor_scalar(mix[:], th[:], 0.5, 0.5,
                                                mybir.AluOpType.mult,
                                                mybir.AluOpType.add)
                        vr_t = pb.tile([128, 128], F32, tag="vrt",
                                       name=f"vr{ch}_{s4}")
                        nc.sync.dma_start(
                            vr_t[:], VR[b, nblk * 128:(nblk + 1) * 128, :])
                        vm128 = pb.tile([128, 128], BF16, tag="vm128",
                                        name=f"vm{ch}_{s4}")
                        for h in range(HL):
                            d1 = pb.tile([128, DH], F32, tag="d1",
                                         name=f"d1{ch}_{s4}_{h}")
                            nc.vector.tensor_sub(d1[:],
                                                 vr_t[:, h * 64:(h + 1) * 64],
                                                 vs[:, h * 64:(h + 1) * 64])
                            d2 = pb.tile([128, DH], F32, tag="d2",
                                         name=f"d2{ch}_{s4}_{h}")
                            nc.vector.tensor_scalar(d2[:], d1[:],
                                                    mix[:, h:h + 1], None,
                                                    mybir.AluOpType.mult)
                            vmt = vm_sb[b][h][:, nblk * 65:nblk * 65 + 64]
                            nc.vector.tensor_add(vmt,
                                                 vs[:, h * 64:(h + 1) * 64],
                                                 d2[:])
                            nc.vector.tensor_copy(
                                vm128[:, h * 64:(h + 1) * 64], vmt)
                        nc.gpsimd.dma_start(
                            V_MIX[b, nblk * 128:(nblk + 1) * 128, :],
                            vm128[:])

                def emit_xlrope(b):
                    for ch4 in range(4):
                        cc = ch4 * 512
                        xrot = pb.tile([128, 512], BF16, tag="xrot",
                                       name=f"xr{b}_{ch4}")
                        for (d, sp) in ROT:
                            nc.sync.dma_start(xrot[d:d + 32, :],
                                              kT_b[b][sp:sp + 32, cc:cc + 512])
                        xa = pb.tile([128, 512], F32, tag="xa",
                                     name=f"xa{b}_{ch4}")
                        nc.vector.tensor_mul(xa[:], kT_b[b][:, cc:cc + 512],
                                             cosk_sb[:, cc:cc + 512])
                        xb = pb.tile([128, 512], F32, tag="xb",
                                     name=f"xb{b}_{ch4}")
                        nc.vector.tensor_mul(xb[:], xrot[:],
                                             sink_sb[:, cc:cc + 512])
                        nc.vector.tensor_add(kT_b[b][:, cc:cc + 512],
                                             xa[:], xb[:])

                # phase 1: batch-0 projections on the full psum layout
                with tc.tile_pool(name="pb_ps", bufs=2, space="PSUM") as bp2, \
                     tc.tile_pool(name="pb_ps1", bufs=1,
                                  space="PSUM") as bp1:
                    def ps1(shape, dt_, tag):
                        if tag in ("qps", "kps"):
                            return bp2.tile(shape, dt_, tag=tag, name=tag)
                        if tag == "vps":
                            return bp2.tile(shape, dt_, tag=tag, name=tag,
                                            bufs=1)
                        return bp1.tile(shape, dt_, tag=tag, name=tag)
                    for ch in range(4):
                        emit_chunk(ch, ps1)
                    emit_xlrope(0)

                # phase 2: batch-0 attention; batch-1 projections squeeze
                # through one shared psum bank in attention's idle cycles
                with tc.tile_pool(name="pc_ps", bufs=1, space="PSUM") as pcp, \
                     tc.tile_pool(name="ab2_ps", bufs=1, space="PSUM") as ab2:
                    def ps2(shape, dt_, tag):
                        return ab2.tile(shape, dt_, tag="ab2", name="ab2")

                    def emit_attention(b):
                        for nt in range(4):
                            n0 = b * N + nt * 512
                            o_ps = [pcp.tile([65, 512], F32,
                                             name=f"oh{h}", tag=f"oh{h}")
                                    for h in range(HL)]
                            for m in range(MT):
                                mlen = 128 if m < 32 else 1
                                s_ps = pcp.tile([128, 1024], F32, tag="s",
                                                name="s", bufs=2)
                                for h in range(HL):
                                    if m < 32:
                                        lk = kT_b[b][h * 64:(h + 1) * 64,
                                                     m * 128:m * 128 + 128]
                                    else:
                                        lk = nullk_sb[h * 64:(h + 1) * 64,
                                                      0:1]
                                    rq = q_T[h * 64:(h + 1) * 64,
                                             n0:n0 + 512]
                                    nc.tensor.matmul(
                                        s_ps[0:mlen, h * 512:(h + 1) * 512],
                                        lk, rq, start=True, stop=True,
                                        tile_position=(h * 64, 0))
                                e = pc.tile([128, 1024], BF16, tag="exps",
                                            name="e", bufs=4)
                                nc.scalar.activation(
                                    e[0:mlen, :], s_ps[0:mlen, :],
                                    mybir.ActivationFunctionType.Exp)
                                for h in range(HL):
                                    if m < 16:
                                        lv = xlv_sb[b][h][0:mlen,
                                                          m * 65:(m + 1) * 65]
                                    elif m < 32:
                                        lv = vm_sb[b][h][0:mlen,
                                                         (m - 16) * 65:
                                                         (m - 15) * 65]
                                    else:
                                        lv = nullva_sb[0:1,
                                                       h * 65:(h + 1) * 65]
                                    nc.tensor.matmul(
                                        o_ps[h][:],
                                        lv, e[0:mlen,
                                              h * 512:(h + 1) * 512],
                                        start=(m == 0), stop=(m == MT - 1),
                                        skip_group_check=True)
                            for h in range(HL):
                                rec = pc.tile([1, 512], F32, tag="rec",
                                              name="rec")
                                nc.vector.reciprocal(rec[:],
                                                     o_ps[h][64:65, :])
                                bc_ps = pcp.tile([64, 512], F32, tag="bc",
                                                 name="bc")
                                nc.tensor.matmul(bc_ps[:],
                                                 ones_f32[0:1, 0:64],
                                                 rec[:], start=True,
                                                 stop=True)
                                bc_sb = pc.tile([64, 512], F32, tag="bcsb",
                                                name="bcsb")
                                nc.vector.tensor_copy(bc_sb[:], bc_ps[:])
                                nc.vector.tensor_mul(
                                    o_norm[h * 64:(h + 1) * 64,
                                           n0:n0 + 512],
                                    o_ps[h][0:64, :], bc_sb[:])
                        for j in range(NC):
                            nc.gpsimd.dma_start(
                                a2a_in[b][j],
                                o_norm[:, b * N + j * 256:
                                       b * N + (j + 1) * 256])
                        nc.gpsimd.collective_compute(
                            "AllToAll", mybir.AluOpType.bypass,
                            replica_groups=[list(range(NC))],
                            ins=[a2a_in[b].ap().opt()],
                            outs=[a2a_out[b].ap().opt()])
                        nc.gpsimd.dma_start(
                            oall_sb[b][:],
                            a2a_out[b].ap().transpose([1, 0, 2]))
                        if dbg:
                            nc.gpsimd.dma_start(
                                DBG_ON[:, b * N:(b + 1) * N],
                                o_norm[:, b * N:(b + 1) * N])

                    emit_attention(0)
                    for ch in range(4, 8):
                        emit_chunk(ch, ps2)
                    emit_xlrope(1)
                    emit_attention(1)
                    for b in range(B):
                        for tch in range(2):
                            for dch in range(2):
                                op = ab2.tile([128, 512], F32, tag="ab2",
                                              name="ab2op")
                                for r in range(NC):
                                    nc.tensor.matmul(
                                        op[:],
                                        oall_sb[b][:, r,
                                                   tch * 128:(tch + 1) * 128],
                                        wout_sb[:, r,
                                                dch * 512:(dch + 1) * 512],
                                        start=(r == 0), stop=(r == NC - 1))
                                os_sb = pc.tile([128, 512], F32, tag="os",
                                                name="os")
                                nc.vector.tensor_copy(os_sb[:], op[:])
                                nc.sync.dma_start(
                                    OUT_SH[b * 256 + tch * 128:
                                           b * 256 + (tch + 1) * 128,
                                           dch * 512:(dch + 1) * 512],
                                    os_sb[:])
    nc.finalize()
    return nc


def _rope_np(pos, t):
    t1, t2 = t[..., :32], t[..., 32:]
    rot = np.concatenate((-t2, t1), axis=-1)
    return t * np.cos(pos) + rot * np.sin(pos)


def _prep(inputs):
    bf = ml_dtypes.bfloat16
    x = np.asarray(inputs["x"], np.float32)
    vr = np.asarray(inputs["value_residual"], np.float32)
    xlm = np.asarray(inputs["xl_memories"], np.float32)
    rq = np.asarray(inputs["rotary_q"], np.float32)
    rk = np.asarray(inputs["rotary_k"], np.float32)
    nw = np.asarray(inputs["norm_w"], np.float32)
    Wq = np.asarray(inputs["Wq"], np.float32)
    Wkv = np.asarray(inputs["Wkv"], np.float32)
    Wout = np.asarray(inputs["Wout"], np.float32)
    nkv = np.asarray(inputs["null_kv"], np.float32)
    Wmix = np.asarray(inputs["Wmix"], np.float32)

    x_t = np.ascontiguousarray(x.reshape(T, DIM).T).astype(bf)

    scale = DH ** -0.5
    Wq_n = nw[:, None] * Wq * scale
    Wk_n = nw[:, None] * Wkv[:, :H * DH]
    Wv_n = nw[:, None] * Wkv[:, H * DH:]
    Wmix_n = nw[:, None] * Wmix

    cq = np.cos(rq).T
    sq = np.sin(rq).T.copy()
    sq[:32] *= -1.0
    cq = np.tile(cq, (2, 1)).astype(np.float16)
    sq = np.tile(sq, (2, 1)).astype(np.float16)

    # kv order on device: [xl (rk[0:M]) | cur (rk[1+M:])]; null uses rk[M]
    pos_k = np.concatenate([rk[0:M], rk[1 + M:1 + M + N]], axis=0)
    ck = np.cos(pos_k).T
    sk = np.sin(pos_k).T.copy()
    sk[:32] *= -1.0
    ck = np.tile(ck, (2, 1)).astype(np.float16)
    sk = np.tile(sk, (2, 1)).astype(np.float16)

    nk_roped = _rope_np(rk[M], nkv[0])     # [16, 64]
    ident = np.eye(128, dtype=np.float32)
    wout_t = np.ascontiguousarray(Wout.reshape(KT, 128, DIM)).astype(bf)

    ones_m = np.ones((B, HL, M, 1), np.float32)
    in_maps = []
    for c in range(NC):
        cols = slice(c * 128, (c + 1) * 128)
        hc = slice(c * HL, c * HL + HL)
        wq_c = np.ascontiguousarray(
            Wq_n[:, cols].reshape(KT, 128, 128)).astype(bf)
        wk_c = np.ascontiguousarray(
            Wk_n[:, cols].reshape(KT, 128, 128)).astype(bf)
        wvm_c = np.ascontiguousarray(
            np.concatenate([Wv_n[:, cols], Wmix_n[:, hc]], axis=1)
            .reshape(KT, 128, 130)).astype(bf)
        xlk_c = np.ascontiguousarray(
            xlm[0][:, hc].transpose(0, 1, 3, 2).reshape(B, 128, M)).astype(bf)
        xlva_c = np.ascontiguousarray(
            np.concatenate([xlm[1][:, hc], ones_m], axis=3)).astype(bf)
        vr_c = np.ascontiguousarray(
            vr[:, hc].transpose(0, 2, 1, 3).reshape(B, N, 128)
        ).astype(np.float32)
        nullk_c = np.ascontiguousarray(nk_roped[hc].reshape(128, 1)).astype(bf)
        nullva_c = np.concatenate(
            [nkv[1][c * HL], [1.0], nkv[1][c * HL + 1], [1.0]]
        ).reshape(1, 130).astype(bf)
        in_maps.append(dict(
            x_t=x_t, wq=wq_c, wk=wk_c, wvm=wvm_c, wout=wout_t,
            cos_q=cq, sin_q=sq, cos_k=ck, sin_k=sk,
            xlk_t=xlk_c, xlva=xlva_c, nullk_t=nullk_c, nullva=nullva_c,
            vr=vr_c, ident_bf=ident.astype(bf), ident_f32=ident))
    return in_maps


def kernel(**inputs):
    if "nc" not in _CACHE:
        _CACHE["nc"] = _build()
    nc = _CACHE["nc"]
    in_maps = _prep(inputs)
    res = run_bass_kernel_spmd(nc, in_maps, core_ids=list(range(NC)))
    rs = res.results

    # out: per batch, core c owns tokens c*256..(c+1)*256
    out = np.empty((B, N, DIM), np.float32)
    for c in range(NC):
        sh = rs[c]["out_shard"]
        for b in range(B):
            out[b, c * 256:(c + 1) * 256] = sh[b * 256:(b + 1) * 256]

    def unpack(name):
        # [B, N, 128] per core -> [B, HL, N, DH] -> concat heads
        parts = [rs[c][name].reshape(B, N, HL, DH).transpose(0, 2, 1, 3)
                 for c in range(NC)]
        return np.concatenate(parts, axis=1)

    k_pre = unpack("k_pre")
    v_mix = unpack("v_mix")
    v_orig = unpack("v_orig")

    nkv = np.asarray(inputs["null_kv"], np.float32)
    nk = np.broadcast_to(nkv[0][None, :, None, :], (B, H, 1, DH))
    nv = np.broadcast_to(nkv[1][None, :, None, :], (B, H, 1, DH))
    next_k = np.concatenate([nk, k_pre], axis=2)
    next_v = np.concatenate([nv, v_mix], axis=2)
    next_xl = np.stack([next_k, next_v]).astype(np.float32)
    return out, next_xl, v_orig.astype(np.float32)
